# revision 1
# baseline (speedup 1.0000x reference)
"""Trainium2 Bass kernel for nn_MessagePassingNN (GNN message passing).

Strategy (8 NeuronCores, SPMD):
  - Nodes sharded: core c owns nodes [c*12500, (c+1)*12500).
  - Edges sharded by owner of `second` (the scatter destination); within a
    core, edges are sorted by `second` and grouped into 128-node destination
    blocks, padded to a fixed number of 128-message tiles per block (TPB).
  - Each iteration: gather h[first] rows from a full DRAM replica
    (indirect DMA), compute msg = selu(main@Wm1 + h[second]@Wm2 + bm) where
    the second-endpoint term is expanded per destination block with a 0/1
    indicator matmul (h[second]@Wm2+bm is computed once per node: C = h@Wm2+bm).
    Segment-sum via indicator matmul into PSUM, GRU update on the core's node
    shard (feature-transposed layout), then AllGather of the updated shard
    into the next replica.
  - Final: per-core partial graph pooling via indicator matmul, AllReduce,
    3-layer MLP (redundantly on every core), output [1, 512] from core 0.

selu(x) = LAM*max(x,0) + min(LAM*ALPHA*exp(x) - LAM*ALPHA, 0), computed as
two halves r and b that are segment-summed separately (PSUM accumulates).
"""
import sys

sys.path.insert(0, "/opt/trn_rl_repo")

import numpy as np

import concourse.bass as bass
import concourse.bacc as bacc
import concourse.mybir as mybir
import concourse.tile as tile
import concourse.bass_utils as bass_utils
from concourse.masks import make_identity

F32 = mybir.dt.float32
I32 = mybir.dt.int32

N_NODES = 100000
N_CORES = 8
SH = N_NODES // N_CORES          # 12500 nodes per core
BLK = 128
NBLK = (SH + BLK - 1) // BLK     # 98 blocks
SHP = NBLK * BLK                 # 12544 padded shard size
H = 128
T_ITERS = 8
G = 512
RU = 256

LAM = 1.0507009873554805
ALPHA = 1.6732632423543772
LA = LAM * ALPHA
LNLA = float(np.log(LA))

AG_GROUPS = [list(range(N_CORES))]
DEBUG_HT = False
ABLATE = set()  # {"gather", "allgather", "msg", "gru", "stage"}
UNROLL = 7
STAGGERED = True
HINTS = (mybir.EngineType.PE, mybir.EngineType.DVE, mybir.EngineType.Activation)


def _sm_row(node):
    """Global node index -> row in the shard-major padded replica."""
    return (node // SH) * SHP + (node % SH)


def _preprocess(features, first, second, graph_ids):
    """Build per-core index/segment arrays. Returns dict of host arrays."""
    first = np.asarray(first, dtype=np.int64)
    second = np.asarray(second, dtype=np.int64)
    graph_ids = np.asarray(graph_ids, dtype=np.int64)
    features = np.asarray(features, dtype=np.float32)

    owner = second // SH
    per_core = []
    counts_all = []
    for c in range(N_CORES):
        m = owner == c
        f_c = first[m]
        loc = (second[m] - c * SH).astype(np.int64)
        order = np.argsort(loc, kind="stable")
        f_c = f_c[order]
        loc = loc[order]
        blk = loc // BLK
        cnt = np.bincount(blk, minlength=NBLK)
        per_core.append((f_c, loc, blk, cnt))
        counts_all.append(cnt)
    counts_all = np.stack(counts_all)
    tpb = int(np.ceil(counts_all.max() / BLK))
    L = NBLK * tpb * BLK  # message slots per core

    gi_list, segc_list, segr_list, gid_list, h0T_list = [], [], [], [], []
    feat_sm = np.zeros((N_CORES * SHP, H), np.float32)
    for c in range(N_CORES):
        feat_sm[c * SHP : c * SHP + SH] = features[c * SH : (c + 1) * SH]
    for c in range(N_CORES):
        f_c, loc, blk, cnt = per_core[c]
        gi = np.zeros(L, np.int32)
        seg = -np.ones(L, np.float32)
        starts = np.concatenate([[0], np.cumsum(cnt)[:-1]])
        within = np.arange(len(loc)) - starts[blk]
        slot = blk * (tpb * BLK) + within
        gi[slot] = _sm_row(f_c).astype(np.int32)
        seg[slot] = (loc - blk * BLK).astype(np.float32)
        gi_list.append(np.ascontiguousarray(gi.reshape(-1, BLK).T))    # [128, NBLK*tpb]
        segc_list.append(np.ascontiguousarray(seg.reshape(-1, BLK).T))  # [128, NBLK*tpb]
        segr_list.append(seg.reshape(1, L).copy())                      # [1, L]
        gid = -np.ones(SHP, np.float32)
        gid[:SH] = graph_ids[c * SH : (c + 1) * SH].astype(np.float32)
        gid_list.append(np.ascontiguousarray(gid.reshape(NBLK, BLK).T))  # [128, NBLK]
        h0T_list.append(np.ascontiguousarray(feat_sm[c * SHP : (c + 1) * SHP].T))

    return dict(
        tpb=tpb,
        L=L,
        feat_sm=feat_sm,
        gi=gi_list,
        segc=segc_list,
        segr=segr_list,
        gid=gid_list,
        h0T=h0T_list,
    )


def _build_program(tpb, b3_val):
    """Build the Bass program. Returns (nc, input_names)."""
    L = NBLK * tpb * BLK
    nc = bacc.Bacc(
        "TRN2",
        target_bir_lowering=False,
        debug=False,
        enable_asserts=False,
        num_devices=N_CORES,
    )

    # --- external tensors ---
    feat_sm = nc.dram_tensor("feat_sm", [N_CORES * SHP, H], F32, kind="ExternalInput")
    h0T_in = nc.dram_tensor("h0T", [BLK, SHP], F32, kind="ExternalInput")
    gi_in = nc.dram_tensor("gi", [BLK, NBLK * tpb], I32, kind="ExternalInput")
    segc_in = nc.dram_tensor("segc", [BLK, NBLK * tpb], F32, kind="ExternalInput")
    segr_in = nc.dram_tensor("segr", [1, L], F32, kind="ExternalInput")
    gid_in = nc.dram_tensor("gid", [BLK, NBLK], F32, kind="ExternalInput")
    wm1_in = nc.dram_tensor("wm1", [H, H], F32, kind="ExternalInput")
    wm2_in = nc.dram_tensor("wm2", [H, H], F32, kind="ExternalInput")
    bmr_in = nc.dram_tensor("bmr", [1, H], F32, kind="ExternalInput")
    wk_in = nc.dram_tensor("wk", [H, 3 * H], F32, kind="ExternalInput")
    uk_in = nc.dram_tensor("uk", [H, 3 * H], F32, kind="ExternalInput")
    bkc_in = nc.dram_tensor("bkc", [BLK, 4], F32, kind="ExternalInput")
    w1_in = nc.dram_tensor("w1", [H, RU], F32, kind="ExternalInput")
    w2_in = nc.dram_tensor("w2", [RU, RU], F32, kind="ExternalInput")
    w3_in = nc.dram_tensor("w3", [RU, 1], F32, kind="ExternalInput")
    b1r_in = nc.dram_tensor("b1r", [BLK, 2], F32, kind="ExternalInput")
    b1e_in = nc.dram_tensor("b1e", [BLK, 2], F32, kind="ExternalInput")
    b2r_in = nc.dram_tensor("b2r", [BLK, 2], F32, kind="ExternalInput")
    b2e_in = nc.dram_tensor("b2e", [BLK, 2], F32, kind="ExternalInput")
    out_dram = nc.dram_tensor("out", [1, G], F32, kind="ExternalOutput")
    dbg_dram = (
        nc.dram_tensor("dbg_hT", [BLK, SHP], F32, kind="ExternalOutput")
        if DEBUG_HT
        else None
    )
    dbg_agg = (
        nc.dram_tensor("dbg_aggT", [BLK, SHP], F32, kind="ExternalOutput")
        if DEBUG_HT
        else None
    )

    with tile.TileContext(nc) as tc:
        with (
            tc.tile_pool(name="const", bufs=1) as cp,
            tc.tile_pool(name="work", bufs=3) as wp,
            tc.tile_pool(name="bigwork", bufs=2) as bwp,
            tc.tile_pool(name="final", bufs=2) as fp,
            tc.tile_pool(name="ps_small", bufs=4, space="PSUM") as psS,
            tc.tile_pool(name="ps_msg", bufs=2, space="PSUM") as psM,
            tc.tile_pool(name="ps_agg", bufs=2, space="PSUM") as psA,
            tc.tile_pool(name="dram", bufs=1, space="DRAM") as dp,
        ):
            # --- resident tiles ---
            ident = cp.tile([BLK, BLK], F32)
            make_identity(nc, ident[:])
            iota_i = cp.tile([BLK, BLK], I32)
            nc.gpsimd.iota(iota_i[:], pattern=[[1, BLK]], base=0, channel_multiplier=0)
            iota_f = cp.tile([BLK, BLK], F32)
            nc.vector.tensor_copy(iota_f[:], iota_i[:])
            iotc_i = cp.tile([BLK, 1], I32)
            nc.gpsimd.iota(iotc_i[:], pattern=[[1, 1]], base=0, channel_multiplier=1)
            iota_col = cp.tile([BLK, 1], F32)
            nc.vector.tensor_copy(iota_col[:], iotc_i[:])
            iog_i = cp.tile([BLK, G], I32)
            nc.gpsimd.iota(iog_i[:], pattern=[[1, G]], base=0, channel_multiplier=0)
            iota_g = cp.tile([BLK, G], F32)
            nc.vector.tensor_copy(iota_g[:], iog_i[:])
            ones1 = cp.tile([1, BLK], F32)
            nc.gpsimd.memset(ones1[:], 1.0)
            lnla_c = cp.tile([BLK, 1], F32)
            nc.gpsimd.memset(lnla_c[:], LNLA)

            hT = cp.tile([BLK, SHP], F32)
            nc.sync.dma_start(hT[:], h0T_in[:])
            gi_sb = cp.tile([BLK, NBLK * tpb], I32)
            nc.sync.dma_start(gi_sb[:], gi_in[:])
            segc_sb = cp.tile([BLK, NBLK * tpb], F32)
            nc.sync.dma_start(segc_sb[:], segc_in[:])
            gid_sb = cp.tile([BLK, NBLK], F32)
            nc.sync.dma_start(gid_sb[:], gid_in[:])

            wm1 = cp.tile([H, H], F32)
            nc.sync.dma_start(wm1[:], wm1_in[:])
            wm2 = cp.tile([H, H], F32)
            nc.sync.dma_start(wm2[:], wm2_in[:])
            bmr = cp.tile([1, H], F32)
            nc.sync.dma_start(bmr[:], bmr_in[:])
            wk = cp.tile([H, 3 * H], F32)
            nc.sync.dma_start(wk[:], wk_in[:])
            uk = cp.tile([H, 3 * H], F32)
            nc.sync.dma_start(uk[:], uk_in[:])
            bkc = cp.tile([BLK, 4], F32)
            nc.sync.dma_start(bkc[:], bkc_in[:])
            w1 = cp.tile([H, RU], F32)
            nc.sync.dma_start(w1[:], w1_in[:])
            w2aa = cp.tile([BLK, BLK], F32)
            nc.sync.dma_start(w2aa[:], w2_in[0:BLK, 0:BLK])
            w2ab = cp.tile([BLK, BLK], F32)
            nc.sync.dma_start(w2ab[:], w2_in[0:BLK, BLK:RU])
            w2ba = cp.tile([BLK, BLK], F32)
            nc.sync.dma_start(w2ba[:], w2_in[BLK:RU, 0:BLK])
            w2bb = cp.tile([BLK, BLK], F32)
            nc.sync.dma_start(w2bb[:], w2_in[BLK:RU, BLK:RU])
            w3a = cp.tile([BLK, 1], F32)
            nc.sync.dma_start(w3a[:], w3_in[0:BLK, :])
            w3b = cp.tile([BLK, 1], F32)
            nc.sync.dma_start(w3b[:], w3_in[BLK:RU, :])
            b1r = cp.tile([BLK, 2], F32)
            nc.sync.dma_start(b1r[:], b1r_in[:])
            b1e = cp.tile([BLK, 2], F32)
            nc.sync.dma_start(b1e[:], b1e_in[:])
            b2r = cp.tile([BLK, 2], F32)
            nc.sync.dma_start(b2r[:], b2r_in[:])
            b2e = cp.tile([BLK, 2], F32)
            nc.sync.dma_start(b2e[:], b2e_in[:])
            b3c = cp.tile([1, 1], F32)
            nc.gpsimd.memset(b3c[:], float(b3_val))

            # --- DRAM scratch ---
            repA = dp.tile([N_CORES * SHP, H], F32)
            repB = dp.tile([N_CORES * SHP, H], F32)
            shard_out = dp.tile([SHP, H], F32)
            pool_in = dp.tile([BLK, G], F32)
            pool_out = dp.tile([BLK, G], F32)

            def block_body(b, src_ap, t):
                # walrus requires a static (physical) offset AP for indirect
                # DMA: copy this block's indices to a fixed tile first.
                # HW consumes ONE offset per partition per call, so issue one
                # indirect gather per 128-message tile.
                idx_blk = wp.tile([BLK, tpb], I32, tag="idx_blk")
                nc.vector.tensor_copy(idx_blk[:], gi_sb[:, bass.ts(b, tpb)])
                gmain = bwp.tile([BLK, tpb * H], F32, tag="gmain")
                if "gather" not in ABLATE:
                    for u in range(1 if "gather1" in ABLATE else tpb):
                        nc.gpsimd.indirect_dma_start(
                            out=gmain[:, bass.ts(u, H)],
                            out_offset=None,
                            in_=src_ap,
                            in_offset=bass.IndirectOffsetOnAxis(
                                ap=idx_blk[:, u : u + 1], axis=0
                            ),
                        )
                # indT[s, m] = (seg[m] == s) for all tpb tiles of this block
                srow = bwp.tile([1, tpb * BLK], F32, tag="srow")
                nc.sync.dma_start(srow[:], segr_in[:1, bass.ts(b, tpb * BLK)])
                segb = bwp.tile([BLK, tpb * BLK], F32, tag="segb")
                nc.gpsimd.partition_broadcast(segb[:], srow[:1, :])
                indT = bwp.tile([BLK, tpb * BLK], F32, tag="indT")
                nc.vector.tensor_scalar(
                    out=indT[:], in0=segb[:], scalar1=iota_col[:, :1], scalar2=None,
                    op0=mybir.AluOpType.is_equal,
                )
                # static-offset copy of this block of hT (walrus: no register
                # offsets on ldweights operands)
                h_in = wp.tile([BLK, BLK], F32, tag="h_in")
                nc.vector.tensor_copy(h_in[:], hT[:, bass.ts(b, BLK)])
                # C = h_blk @ Wm2 + bm  (node-term of the message MLP)
                c_ps = psS.tile([BLK, H], F32, space="PSUM", tag="ps")
                nc.tensor.matmul(c_ps[:], lhsT=h_in[:], rhs=wm2[:],
                                 start=True, stop=False)
                nc.tensor.matmul(c_ps[:], lhsT=ones1[:], rhs=bmr[:],
                                 start=False, stop=True)
                c_sb = wp.tile([BLK, H], F32, tag="c_sb")
                nc.scalar.copy(c_sb[:], c_ps[:])

                aggT_ps = psA.tile([BLK, BLK], F32, space="PSUM", tag="aggT")
                for u in range(1) if "msg1" in ABLATE else ([] if "msg" in ABLATE else range(tpb)):
                    mt_ps = psS.tile([BLK, BLK], F32, space="PSUM", tag="ps")
                    nc.tensor.transpose(mt_ps[:], gmain[:, bass.ts(u, H)], ident[:])
                    mainT = wp.tile([BLK, BLK], F32, tag="mainT")
                    nc.vector.tensor_copy(mainT[:], mt_ps[:])
                    msg_ps = psM.tile([BLK, H], F32, space="PSUM", tag="msg")
                    nc.tensor.matmul(msg_ps[:], lhsT=mainT[:], rhs=wm1[:],
                                     start=True, stop=False)
                    nc.tensor.matmul(msg_ps[:], lhsT=indT[:, bass.ts(u, BLK)],
                                     rhs=c_sb[:], start=False, stop=True)
                    r_t = wp.tile([BLK, H], F32, tag="r_t")
                    nc.scalar.activation(
                        r_t[:], msg_ps[:], mybir.ActivationFunctionType.Relu,
                        scale=LAM,
                    )
                    e2_t = wp.tile([BLK, H], F32, tag="e2_t")
                    nc.scalar.activation(
                        e2_t[:], msg_ps[:], mybir.ActivationFunctionType.Exp,
                        bias=lnla_c[:, :1], scale=1.0,
                    )
                    b_t = wp.tile([BLK, H], F32, tag="b_t")
                    nc.vector.tensor_scalar(
                        out=b_t[:], in0=e2_t[:], scalar1=LA, scalar2=0.0,
                        op0=mybir.AluOpType.subtract, op1=mybir.AluOpType.min,
                    )
                    ind_ms = wp.tile([BLK, BLK], F32, tag="ind_ms")
                    nc.vector.tensor_scalar(
                        out=ind_ms[:], in0=iota_f[:],
                        scalar1=segc_sb[:, bass.ds(b * tpb + u, 1)], scalar2=None,
                        op0=mybir.AluOpType.is_equal,
                    )
                    # aggT[j, s] += r.T-free matmul: lhsT=r [m, j], rhs=ind [m, s]
                    nc.tensor.matmul(aggT_ps[:], lhsT=r_t[:], rhs=ind_ms[:],
                                     start=(u == 0), stop=False)
                    nc.tensor.matmul(aggT_ps[:], lhsT=b_t[:], rhs=ind_ms[:],
                                     start=False, stop=(u == tpb - 1))

                aggT = wp.tile([BLK, BLK], F32, tag="aggT_sb")
                nc.scalar.copy(aggT[:], aggT_ps[:])
                if dbg_agg is not None and t == 0:
                    nc.sync.dma_start(dbg_agg[:, bass.ts(b, BLK)], aggT[:])

                # --- GRU (feature-transposed layout [l, n]) ---
                h_blk = h_in[:]
                mz_ps = psS.tile([BLK, BLK], F32, space="PSUM", tag="ps")
                nc.tensor.matmul(mz_ps[:], lhsT=wk[:, 0:H], rhs=aggT[:],
                                 start=True, stop=False)
                nc.tensor.matmul(mz_ps[:], lhsT=uk[:, 0:H], rhs=h_blk,
                                 start=False, stop=True)
                zT = wp.tile([BLK, BLK], F32, tag="zT")
                nc.scalar.activation(zT[:], mz_ps[:],
                                     mybir.ActivationFunctionType.Sigmoid,
                                     bias=bkc[:, 0:1])
                mr_ps = psS.tile([BLK, BLK], F32, space="PSUM", tag="ps")
                nc.tensor.matmul(mr_ps[:], lhsT=wk[:, H : 2 * H], rhs=aggT[:],
                                 start=True, stop=False)
                nc.tensor.matmul(mr_ps[:], lhsT=uk[:, H : 2 * H], rhs=h_blk,
                                 start=False, stop=True)
                rT = wp.tile([BLK, BLK], F32, tag="rT")
                nc.scalar.activation(rT[:], mr_ps[:],
                                     mybir.ActivationFunctionType.Sigmoid,
                                     bias=bkc[:, 1:2])
                mhx_ps = psS.tile([BLK, BLK], F32, space="PSUM", tag="ps")
                nc.tensor.matmul(mhx_ps[:], lhsT=wk[:, 2 * H : 3 * H], rhs=aggT[:],
                                 start=True, stop=True)
                mhh_ps = psS.tile([BLK, BLK], F32, space="PSUM", tag="ps")
                nc.tensor.matmul(mhh_ps[:], lhsT=uk[:, 2 * H : 3 * H], rhs=h_blk,
                                 start=True, stop=True)
                t1 = wp.tile([BLK, BLK], F32, tag="t1")
                nc.vector.tensor_scalar(out=t1[:], in0=mhh_ps[:],
                                        scalar1=bkc[:, 3:4], scalar2=None,
                                        op0=mybir.AluOpType.add)
                t2 = wp.tile([BLK, BLK], F32, tag="t2")
                nc.vector.tensor_tensor(out=t2[:], in0=t1[:], in1=rT[:],
                                        op=mybir.AluOpType.mult)
                t3 = wp.tile([BLK, BLK], F32, tag="t3")
                nc.vector.tensor_tensor(out=t3[:], in0=t2[:], in1=mhx_ps[:],
                                        op=mybir.AluOpType.add)
                hhT = wp.tile([BLK, BLK], F32, tag="hhT")
                nc.scalar.activation(hhT[:], t3[:],
                                     mybir.ActivationFunctionType.Tanh,
                                     bias=bkc[:, 2:3])
                d_t = wp.tile([BLK, BLK], F32, tag="d_t")
                nc.vector.tensor_tensor(out=d_t[:], in0=h_blk, in1=hhT[:],
                                        op=mybir.AluOpType.subtract)
                e_t = wp.tile([BLK, BLK], F32, tag="e_t")
                nc.vector.tensor_tensor(out=e_t[:], in0=zT[:], in1=d_t[:],
                                        op=mybir.AluOpType.mult)
                hnT = wp.tile([BLK, BLK], F32, tag="hnT")
                nc.vector.tensor_tensor(out=hnT[:], in0=hhT[:], in1=e_t[:],
                                        op=mybir.AluOpType.add)
                nc.vector.tensor_copy(hT[:, bass.ts(b, BLK)], hnT[:])

                if t < T_ITERS - 1 and "stage" not in ABLATE:
                    hn_ps = psS.tile([BLK, BLK], F32, space="PSUM", tag="ps")
                    nc.tensor.transpose(hn_ps[:], hnT[:], ident[:])
                    hn_sb = wp.tile([BLK, BLK], F32, tag="hn_sb")
                    nc.scalar.copy(hn_sb[:], hn_ps[:])
                    nc.sync.dma_start(shard_out[bass.ts(b, BLK), :], hn_sb[:])

            # --- main iterations ---
            for t in range(T_ITERS):
                if t == 0:
                    src_ap = feat_sm[:]
                elif t % 2 == 1:
                    src_ap = repA[:]
                else:
                    src_ap = repB[:]
                with tc.For_i(
                    0, NBLK, UNROLL,
                    staggered_reset=STAGGERED,
                    hint_engines=HINTS,
                ) as b:
                    for db in range(UNROLL):
                        block_body(b + db if db else b, src_ap, t)
                if t < T_ITERS - 1 and "allgather" not in ABLATE:
                    dst = repA if t % 2 == 0 else repB
                    nc.gpsimd.collective_compute(
                        "AllGather",
                        mybir.AluOpType.bypass,
                        replica_groups=AG_GROUPS,
                        ins=[shard_out.opt()],
                        outs=[dst.opt()],
                    )

            if dbg_dram is not None:
                nc.sync.dma_start(dbg_dram[:], hT[:])

            # --- graph pooling: pooledT[j, g] = sum_s h[s, j] * (gid[s] == g) ---
            pool_ps = psM.tile([BLK, G], F32, space="PSUM", tag="msg")
            for b in range(NBLK):
                hb_ps = psS.tile([BLK, BLK], F32, space="PSUM", tag="ps")
                nc.tensor.transpose(hb_ps[:], hT[:, bass.ts(b, BLK)], ident[:])
                hb_sb = fp.tile([BLK, BLK], F32, tag="hb_sb")
                nc.scalar.copy(hb_sb[:], hb_ps[:])
                indg = fp.tile([BLK, G], F32, tag="indg")
                nc.vector.tensor_scalar(
                    out=indg[:], in0=iota_g[:], scalar1=gid_sb[:, b : b + 1],
                    scalar2=None, op0=mybir.AluOpType.is_equal,
                )
                nc.tensor.matmul(pool_ps[:], lhsT=hb_sb[:], rhs=indg[:],
                                 start=(b == 0), stop=(b == NBLK - 1))
            pooledT = fp.tile([BLK, G], F32, tag="pooledT")
            nc.vector.tensor_copy(pooledT[:], pool_ps[:])
            nc.sync.dma_start(pool_in[:], pooledT[:])
            nc.gpsimd.collective_compute(
                "AllReduce",
                mybir.AluOpType.add,
                replica_groups=AG_GROUPS,
                ins=[pool_in.opt()],
                outs=[pool_out.opt()],
            )
            pld = fp.tile([BLK, G], F32, tag="pld")
            nc.sync.dma_start(pld[:], pool_out[:])

            # --- MLP ---
            def selu_block(x_ps, brel_col, bexp_col, tagp):
                rr = fp.tile([BLK, G], F32, tag="f_r")
                nc.scalar.activation(rr[:], x_ps[:],
                                     mybir.ActivationFunctionType.Relu,
                                     bias=brel_col, scale=LAM)
                ee = fp.tile([BLK, G], F32, tag="f_e")
                nc.scalar.activation(ee[:], x_ps[:],
                                     mybir.ActivationFunctionType.Exp,
                                     bias=bexp_col, scale=1.0)
                bb = fp.tile([BLK, G], F32, tag="f_b")
                nc.vector.tensor_scalar(
                    out=bb[:], in0=ee[:], scalar1=LA, scalar2=0.0,
                    op0=mybir.AluOpType.subtract, op1=mybir.AluOpType.min,
                )
                oo = fp.tile([BLK, G], F32, tag=f"{tagp}_o")
                nc.vector.tensor_tensor(out=oo[:], in0=rr[:], in1=bb[:],
                                        op=mybir.AluOpType.add)
                return oo

            x1 = []
            for half in range(2):
                x_ps = psM.tile([BLK, G], F32, space="PSUM", tag="msg")
                nc.tensor.matmul(x_ps[:], lhsT=w1[:, bass.ts(half, BLK)], rhs=pld[:],
                                 start=True, stop=True)
                x1.append(selu_block(x_ps, b1r[:, half : half + 1],
                                     b1e[:, half : half + 1], f"x1{half}"))
            x2 = []
            w2t = [[w2aa, w2ab], [w2ba, w2bb]]
            for half in range(2):
                x_ps = psM.tile([BLK, G], F32, space="PSUM", tag="msg")
                nc.tensor.matmul(x_ps[:], lhsT=w2t[0][half][:], rhs=x1[0][:],
                                 start=True, stop=False)
                nc.tensor.matmul(x_ps[:], lhsT=w2t[1][half][:], rhs=x1[1][:],
                                 start=False, stop=True)
                x2.append(selu_block(x_ps, b2r[:, half : half + 1],
                                     b2e[:, half : half + 1], f"x2{half}"))
            x3_ps = psS.tile([1, G], F32, space="PSUM", tag="ps")
            nc.tensor.matmul(x3_ps[:], lhsT=w3a[:], rhs=x2[0][:],
                             start=True, stop=False)
            nc.tensor.matmul(x3_ps[:], lhsT=w3b[:], rhs=x2[1][:],
                             start=False, stop=True)
            out_sb = fp.tile([1, G], F32, tag="out_sb")
            nc.scalar.activation(out_sb[:], x3_ps[:],
                                 mybir.ActivationFunctionType.Identity,
                                 bias=b3c[:1, :1])
            nc.sync.dma_start(out_dram[:], out_sb[:])

    nc.compile()
    return nc


def kernel(features, edges_topology, graph_ids, Wm, bm, Wk, Uk, bk,
           W1, b1, W2, b2, W3, b3, _trace=False):
    features = np.asarray(features, np.float32)
    Wm = np.asarray(Wm, np.float32)
    bm = np.asarray(bm, np.float32)
    Wk = np.asarray(Wk, np.float32)
    Uk = np.asarray(Uk, np.float32)
    bk = np.asarray(bk, np.float32)
    W1 = np.asarray(W1, np.float32)
    b1 = np.asarray(b1, np.float32)
    W2 = np.asarray(W2, np.float32)
    b2 = np.asarray(b2, np.float32)
    W3 = np.asarray(W3, np.float32)
    b3 = np.asarray(b3, np.float32)
    et = np.asarray(edges_topology)

    pp = _preprocess(features, et[0], et[1], graph_ids)
    tpb = pp["tpb"]

    nc = _build_program(tpb, float(b3[0]))

    bkc = np.stack(
        [
            bk[0, 0:H] + bk[1, 0:H],
            bk[0, H : 2 * H] + bk[1, H : 2 * H],
            bk[0, 2 * H : 3 * H],
            bk[1, 2 * H : 3 * H],
        ],
        axis=1,
    ).astype(np.float32)  # [128, 4]: bz, br, bhx, bhh

    b1r = np.stack([LAM * b1[0:BLK], LAM * b1[BLK:RU]], axis=1).astype(np.float32)
    b1e = np.stack([b1[0:BLK] + LNLA, b1[BLK:RU] + LNLA], axis=1).astype(np.float32)
    b2r = np.stack([LAM * b2[0:BLK], LAM * b2[BLK:RU]], axis=1).astype(np.float32)
    b2e = np.stack([b2[0:BLK] + LNLA, b2[BLK:RU] + LNLA], axis=1).astype(np.float32)

    in_maps = []
    for c in range(N_CORES):
        in_maps.append(
            {
                "feat_sm": pp["feat_sm"],
                "h0T": pp["h0T"][c],
                "gi": pp["gi"][c],
                "segc": pp["segc"][c],
                "segr": pp["segr"][c],
                "gid": pp["gid"][c],
                "wm1": np.ascontiguousarray(Wm[0:H]),
                "wm2": np.ascontiguousarray(Wm[H : 2 * H]),
                "bmr": bm.reshape(1, H),
                "wk": Wk,
                "uk": Uk,
                "bkc": bkc,
                "w1": W1,
                "w2": W2,
                "w3": W3,
                "b1r": b1r,
                "b1e": b1e,
                "b2r": b2r,
                "b2e": b2e,
            }
        )

    res = bass_utils.run_bass_kernel_spmd(
        nc, in_maps, core_ids=list(range(N_CORES)), trace=_trace
    )
    out = res.results[0]["out"].reshape(G, 1).astype(np.float32)
    kernel.last_results = res
    return out



# revision 25
# speedup vs baseline: 4.6503x; 4.6503x over previous
"""Trainium2 Bass kernel for nn_MessagePassingNN (GNN message passing).

Strategy (8 NeuronCores, SPMD), v2 -- see problem notes:
  - Nodes sharded by index range: core c owns nodes [c*12500, (c+1)*12500).
  - Per iteration each core computes per-node terms A = h@Wm1, C = h@Wm2+bm
    (bf16) for its shard, AllGathers A into a bf16 replica viewed as
    [25088, 512] (4 node-rows per 1024B physical row).
  - Messages msg = selu(A[first] + C[second]): edges grouped by destination
    128-node block and source-row residue q = row%4 (int16 dma_gather limit);
    each (block, q) run padded to 128-message tiles with a cross-core-uniform
    tile pattern P[b, q].  A[first] via one dma_gather per (4-block group, q);
    C[second] expanded per tile by an indicator matmul (indT, host-streamed);
    A added by identity matmul into the same PSUM bank (4 tiles per bank).
    selu = relu part (scalar engine) + exp part (scalar + vector min); both
    segment-summed into the group's agg PSUM bank via indicator matmuls
    (indm, host-streamed).
  - GRU per block (feature-transposed, bf16 matmuls / fp32 state), then A/C
    for the next iteration immediately.
  - Final: indicator-matmul graph pooling, AllReduce, 3-layer MLP, out[512,1].
"""
import sys

sys.path.insert(0, "/opt/trn_rl_repo")

import numpy as np
import ml_dtypes

import concourse.bass as bass
import concourse.bacc as bacc
import concourse.mybir as mybir
import concourse.tile as tile
import concourse.bass_utils as bass_utils
from concourse.masks import make_identity

F32 = mybir.dt.float32
BF16 = mybir.dt.bfloat16
F8 = mybir.dt.float8e4
I32 = mybir.dt.int32
I16 = mybir.dt.int16

N_NODES = 100000
N_CORES = 8
SH = N_NODES // N_CORES          # 12500
BLK = 128
NBLK = (SH + BLK - 1) // BLK     # 98
SHP = NBLK * BLK                 # 12544
H = 128
T_ITERS = 8
G = 512
RU = 256
NQ = 4
QROWS = (N_CORES * SHP) // NQ    # 25088
GB = 4                           # blocks per gather group
NGRP = (NBLK + GB - 1) // GB     # 25 groups (last has 2 blocks)

LAM = 1.0507009873554805
ALPHA = 1.6732632423543772
LA = LAM * ALPHA
LNLA = float(np.log(LA))

AG_GROUPS = [list(range(N_CORES))]


def _sm_row(node):
    node = np.asarray(node)
    return (node // SH) * SHP + (node % SH)


def _pack_core(deg4, rng):
    """Assign SH nodes (class = original position % NQ, fixed) to blocks so
    per-(block, src-residue) in-edge counts fit the alternating pattern
    P[b, q] = 2 + ((b + q) % 2) tiles.  Returns pos[n] = permuted position,
    preserving pos % NQ == n % NQ.  Greedy; degree caps may be exceeded (the
    caller recomputes the realized pattern)."""
    capP = np.array([[2 + ((b + q) % 2) for q in range(NQ)]
                     for b in range(NBLK)], np.int64) * BLK
    cls = np.arange(SH) % NQ
    cls_slots = np.full((NBLK, NQ), BLK // NQ, np.int64)
    cls_slots[-1] = (SH - BLK * (NBLK - 1)) // NQ        # 21 per class
    fill = np.zeros((NBLK, NQ), np.int64)
    order = np.argsort(-deg4.sum(1), kind="stable")
    assign = np.full(SH, -1, np.int64)
    BIG = 1 << 40
    for n in order:
        c = cls[n]
        d = deg4[n]
        rem = capP - fill - d                      # [NBLK, NQ]
        over = np.where(rem < 0, -rem, 0).sum(1)   # overflow if placed here
        room = rem.min(1)
        score = over * 4096 - room
        score[cls_slots[:, c] <= 0] = BIG
        b = int(np.argmin(score))
        fill[b] += d
        cls_slots[b, c] -= 1
        assign[n] = b
    # positions: class-c nodes of block b -> slots b*BLK + c, c+4, ...
    pos = np.full(SH, -1, np.int64)
    nxt = np.zeros((NBLK, NQ), np.int64)
    for n in range(SH):
        b, c = assign[n], cls[n]
        pos[n] = b * BLK + c + NQ * nxt[b, c]
        nxt[b, c] += 1
    return pos


def _preprocess(features, first, second, graph_ids):
    first = np.asarray(first, dtype=np.int64)
    second = np.asarray(second, dtype=np.int64)
    graph_ids = np.asarray(graph_ids, dtype=np.int64)
    features = np.asarray(features, dtype=np.float32)
    rng = np.random.default_rng(0)

    owner_src = first // SH
    owner = second // SH
    src_q0 = (first % SH) % NQ          # class: preserved by the permutation

    # per-core degree-aware packing (class-preserving permutation)
    pos_list = []
    for c in range(N_CORES):
        m = owner == c
        loc = second[m] - c * SH
        d4 = np.zeros((SH, NQ), np.int64)
        np.add.at(d4, (loc, src_q0[m]), 1)
        pos_list.append(_pack_core(d4, rng))

    # permuted source rows (per owner) -> gather idx and residue
    pos_all = np.concatenate(
        [o * SHP + pos_list[o] for o in range(N_CORES)])   # [N_NODES]
    src_row = pos_all[first]            # permuted sm_row of source
    src_q = (src_row % NQ).astype(np.int64)
    src_idx = (src_row // NQ).astype(np.int64)     # < 25088, int16-safe
    assert (src_q == src_q0).all()

    per_core = []
    counts = np.zeros((N_CORES, NBLK, NQ), np.int64)
    for c in range(N_CORES):
        m = owner == c
        loc = pos_list[c][second[m] - c * SH]
        blk = loc // BLK
        q = src_q[m]
        si = src_idx[m]
        order = np.lexsort((si, q, blk))
        per_core.append((loc[order], blk[order], q[order], si[order]))
        np.add.at(counts[c], (blk, q), 1)

    P = np.maximum(1, -(-counts.max(axis=0) // BLK))      # [NBLK, NQ] tiles

    groups = [list(range(g * GB, min((g + 1) * GB, NBLK))) for g in range(NGRP)]
    tile_block = []
    call_sizes = []
    for blocks in groups:
        for q in range(NQ):
            csz = 0
            for b in blocks:
                for _ in range(int(P[b, q])):
                    tile_block.append(b)
                    csz += BLK
            call_sizes.append(csz)
    TT = len(tile_block)
    LC = TT * BLK
    tile_block = np.array(tile_block, np.int64)
    first_tile = np.full(NBLK, -1, np.int64)
    last_tile = np.zeros(NBLK, np.int64)
    for t, b in enumerate(tile_block):
        if first_tile[b] < 0:
            first_tile[b] = t
        last_tile[b] = t

    # slot base per (b, q)
    slot_base = np.zeros((NBLK, NQ), np.int64)
    s = 0
    for blocks in groups:
        for q in range(NQ):
            for b in blocks:
                slot_base[b, q] = s
                s += int(P[b, q]) * BLK
    assert s == LC

    idx16_list, indT_list, indm_list = [], [], []
    h0T_list, gid_list = [], []
    for c in range(N_CORES):
        loc, blk, q, sidx = per_core[c]
        seg = np.full(LC, -1.0, np.float32)
        gidx = np.zeros(LC, np.int64)
        if len(blk):
            run_change = np.ones(len(blk), bool)
            run_change[1:] = (blk[1:] != blk[:-1]) | (q[1:] != q[:-1])
            run_start = np.maximum.accumulate(
                np.where(run_change, np.arange(len(blk)), 0))
            within = np.arange(len(blk)) - run_start
            slot = slot_base[blk, q] + within
            seg[slot] = (loc - blk * BLK).astype(np.float32)
            gidx[slot] = sidx
        idx16 = gidx.astype(np.int16)
        idxw = np.zeros((16, LC // 16), np.int16)
        s0 = 0
        for csz in call_sizes:
            if csz:
                idxw[:, s0 // 16 : (s0 + csz) // 16] = (
                    idx16[s0 : s0 + csz].reshape(csz // 16, 16).T)
                s0 += csz
        idx_full = np.ascontiguousarray(np.tile(idxw, (8, 1)))
        segs = seg.reshape(TT, 1, BLK)
        dd = np.arange(BLK, dtype=np.float32).reshape(1, BLK, 1)
        indT = (segs == dd).astype(ml_dtypes.float8_e4m3)   # [TT, dest, msg]
        indT_h = np.ascontiguousarray(
            indT.transpose(1, 0, 2).reshape(BLK, LC))
        indm = indT.transpose(0, 2, 1)                   # [TT, msg, dest]
        indm_h = np.ascontiguousarray(
            indm.transpose(1, 0, 2).reshape(BLK, LC))
        idx16_list.append(idx_full)
        indT_list.append(indT_h)
        indm_list.append(indm_h)
        h0 = np.zeros((SHP, H), np.float32)
        h0[pos_list[c]] = features[c * SH : (c + 1) * SH]
        h0T_list.append(np.ascontiguousarray(h0.T))
        gid = np.full(SHP, -1.0, np.float32)
        gid[pos_list[c]] = graph_ids[c * SH : (c + 1) * SH].astype(np.float32)
        gid_list.append(np.ascontiguousarray(gid.reshape(NBLK, BLK).T))

    return dict(
        P=P, groups=groups, TT=TT, LC=LC, call_sizes=call_sizes,
        tile_block=tile_block, first_tile=first_tile, last_tile=last_tile,
        tile_base=slot_base // BLK,
        idx16=idx16_list, indT=indT_list, indm=indm_list,
        h0T=h0T_list, gid=gid_list,
    )


def _build_program(meta, b3_val, t_iters=T_ITERS):
    groups = meta["groups"]
    TT = meta["TT"]
    LC = meta["LC"]
    call_sizes = meta["call_sizes"]
    tile_block = meta["tile_block"]
    tile_base = meta["tile_base"]
    P = meta["P"]

    nc = bacc.Bacc(
        "TRN2",
        target_bir_lowering=False,
        debug=False,
        enable_asserts=False,
        num_devices=N_CORES,
    )

    h0T_in = nc.dram_tensor("h0T", [BLK, SHP], F32, kind="ExternalInput")
    idx_in = nc.dram_tensor("idx16", [BLK, LC // 16], I16, kind="ExternalInput")
    indT_in = nc.dram_tensor("indT", [BLK, LC], F8, kind="ExternalInput")
    indm_in = nc.dram_tensor("indm", [BLK, LC], F8, kind="ExternalInput")
    gid_in = nc.dram_tensor("gid", [BLK, NBLK], F32, kind="ExternalInput")
    wm1_in = nc.dram_tensor("wm1", [H, H], F32, kind="ExternalInput")
    wm2_in = nc.dram_tensor("wm2", [H, H], F32, kind="ExternalInput")
    bmr_in = nc.dram_tensor("bmr", [1, H], F32, kind="ExternalInput")
    wk_in = nc.dram_tensor("wk", [H, 3 * H], F32, kind="ExternalInput")
    uk_in = nc.dram_tensor("uk", [H, 3 * H], F32, kind="ExternalInput")
    bkc_in = nc.dram_tensor("bkc", [BLK, 4], F32, kind="ExternalInput")
    w1_in = nc.dram_tensor("w1", [H, RU], F32, kind="ExternalInput")
    w2_in = nc.dram_tensor("w2", [RU, RU], F32, kind="ExternalInput")
    w3_in = nc.dram_tensor("w3", [RU, 1], F32, kind="ExternalInput")
    b1r_in = nc.dram_tensor("b1r", [BLK, 2], F32, kind="ExternalInput")
    b1e_in = nc.dram_tensor("b1e", [BLK, 2], F32, kind="ExternalInput")
    b2r_in = nc.dram_tensor("b2r", [BLK, 2], F32, kind="ExternalInput")
    b2e_in = nc.dram_tensor("b2e", [BLK, 2], F32, kind="ExternalInput")
    out_dram = nc.dram_tensor("out", [1, G], F32, kind="ExternalOutput")

    # per-group slot ranges
    grp_slot0, grp_ntiles = [], []
    s0 = 0
    ci = 0
    for blocks in groups:
        n = 0
        for q in range(NQ):
            n += call_sizes[ci]
            ci += 1
        grp_slot0.append(s0)
        grp_ntiles.append(n // BLK)
        s0 += n

    with tile.TileContext(nc) as tc:
        with (
            tc.tile_pool(name="const", bufs=1) as cp,
            tc.tile_pool(name="gmp", bufs=2) as gmp,
            tc.tile_pool(name="indp", bufs=2) as indp,
            tc.tile_pool(name="sp", bufs=3) as spool,
            tc.tile_pool(name="wp", bufs=2) as wp,
            tc.tile_pool(name="final", bufs=1) as fp,
            tc.tile_pool(name="ps_x", bufs=2, space="PSUM") as psX,
            tc.tile_pool(name="ps_agg", bufs=2, space="PSUM") as psA,
            tc.tile_pool(name="ps_gru", bufs=2, space="PSUM") as psG,
            tc.tile_pool(name="ps_ac", bufs=2, space="PSUM") as psC,
            tc.tile_pool(name="dram", bufs=1, space="DRAM") as dp,
        ):
            ident_b = cp.tile([BLK, BLK], BF16)
            make_identity(nc, ident_b[:])
            ident_f = cp.tile([BLK, BLK], F32)
            make_identity(nc, ident_f[:])
            iota_g = cp.tile([BLK, G], F32)
            lnla_c = cp.tile([BLK, 1], F32)
            nc.gpsimd.memset(lnla_c[:], LNLA)
            ones1_b = cp.tile([1, BLK], BF16)
            nc.gpsimd.memset(ones1_b[:], 1.0)

            iog_i = wp.tile([BLK, G], I32, tag="iogi")
            nc.gpsimd.iota(iog_i[:], pattern=[[1, G]], base=0,
                           channel_multiplier=0)
            nc.vector.tensor_copy(iota_g[:], iog_i[:])
            hT = cp.tile([BLK, SHP], F32)
            nc.sync.dma_start(hT[:], h0T_in[:])
            idx_sb = cp.tile([BLK, LC // 16], I16)
            nc.sync.dma_start(idx_sb[:], idx_in[:])
            gid_sb = cp.tile([BLK, NBLK], F32)
            nc.sync.dma_start(gid_sb[:], gid_in[:])

            def load_bf(t_in, shape, tag):
                t32 = wp.tile(shape, F32, tag="ldf32")
                nc.sync.dma_start(t32[:], t_in[:])
                tb = cp.tile(shape, BF16, tag=tag)
                nc.vector.tensor_copy(tb[:], t32[:])
                return tb

            wm1 = load_bf(wm1_in, [H, H], "wm1")
            wm2 = load_bf(wm2_in, [H, H], "wm2")
            wk = load_bf(wk_in, [H, 3 * H], "wk")
            uk = load_bf(uk_in, [H, 3 * H], "uk")
            w1 = load_bf(w1_in, [H, RU], "w1")
            bmr_b = load_bf(bmr_in, [1, H], "bmr")
            bkc = cp.tile([BLK, 4], F32)
            nc.sync.dma_start(bkc[:], bkc_in[:])
            w2q = []
            for i in range(2):
                row = []
                for j in range(2):
                    t32 = wp.tile([BLK, BLK], F32, tag="ldw2")
                    nc.sync.dma_start(
                        t32[:], w2_in[bass.ts(i, BLK), bass.ts(j, BLK)])
                    tb = cp.tile([BLK, BLK], BF16, tag=f"w2q{i}{j}")
                    nc.vector.tensor_copy(tb[:], t32[:])
                    row.append(tb)
                w2q.append(row)
            w3ab = []
            for i in range(2):
                t32 = wp.tile([BLK, 1], F32, tag="ldw3")
                nc.sync.dma_start(t32[:], w3_in[bass.ts(i, BLK), :])
                tb = cp.tile([BLK, 1], BF16, tag=f"w3{i}")
                nc.vector.tensor_copy(tb[:], t32[:])
                w3ab.append(tb)
            w3a, w3b = w3ab
            b1r = cp.tile([BLK, 2], F32)
            nc.sync.dma_start(b1r[:], b1r_in[:])
            b1e = cp.tile([BLK, 2], F32)
            nc.sync.dma_start(b1e[:], b1e_in[:])
            b2r = cp.tile([BLK, 2], F32)
            nc.sync.dma_start(b2r[:], b2r_in[:])
            b2e = cp.tile([BLK, 2], F32)
            nc.sync.dma_start(b2e[:], b2e_in[:])
            b3c = cp.tile([1, 1], F32)
            nc.gpsimd.memset(b3c[:], float(b3_val))

            c_all = cp.tile([BLK, NBLK * H], BF16)

            repA = dp.tile([QROWS, NQ * H], BF16)
            repB = dp.tile([QROWS, NQ * H], BF16)
            shardA = dp.tile([SHP, H], BF16)
            pool_in = dp.tile([BLK, G], F32)
            pool_out = dp.tile([BLK, G], F32)

            def ac_block(b):
                h_bf = wp.tile([BLK, BLK], BF16, tag="h_bf")
                nc.vector.tensor_copy(h_bf[:], hT[:, bass.ts(b, BLK)])
                ac_ps = psC.tile([BLK, 2 * H], F32, space="PSUM", tag="ac")
                nc.tensor.matmul(ac_ps[:, 0:H], lhsT=h_bf[:], rhs=wm1[:],
                                 start=True, stop=True)
                nc.tensor.matmul(ac_ps[:, H : 2 * H], lhsT=h_bf[:], rhs=wm2[:],
                                 start=True, stop=False)
                nc.tensor.matmul(ac_ps[:, H : 2 * H], lhsT=ones1_b[:],
                                 rhs=bmr_b[:], start=False, stop=True)
                a_sb = wp.tile([BLK, H], BF16, tag="a_sb")
                nc.vector.tensor_copy(a_sb[:], ac_ps[:, 0:H])
                nc.sync.dma_start(shardA[bass.ts(b, BLK), :], a_sb[:])
                nc.vector.tensor_copy(c_all[:, bass.ts(b, H)],
                                      ac_ps[:, H : 2 * H])

            def gru_block(b, agg_ap, t):
                # sigmoid(x) = (tanh(x/2) + 1)/2 so every activation comes
                # from the exp_and_others table set (no table reloads).
                aggb = wp.tile([BLK, BLK], BF16, tag="aggb")
                nc.vector.tensor_copy(aggb[:], agg_ap)
                h_bf = wp.tile([BLK, BLK], BF16, tag="hgru")
                nc.vector.tensor_copy(h_bf[:], hT[:, bass.ts(b, BLK)])
                gps = psG.tile([BLK, 4 * H], F32, space="PSUM", tag="gru")
                nc.tensor.matmul(gps[:, 0:H], lhsT=wk[:, 0:H], rhs=aggb[:],
                                 start=True, stop=False)
                nc.tensor.matmul(gps[:, 0:H], lhsT=uk[:, 0:H], rhs=h_bf[:],
                                 start=False, stop=True)
                nc.tensor.matmul(gps[:, H : 2 * H], lhsT=wk[:, H : 2 * H],
                                 rhs=aggb[:], start=True, stop=False)
                nc.tensor.matmul(gps[:, H : 2 * H], lhsT=uk[:, H : 2 * H],
                                 rhs=h_bf[:], start=False, stop=True)
                nc.tensor.matmul(gps[:, 2 * H : 3 * H],
                                 lhsT=wk[:, 2 * H : 3 * H], rhs=aggb[:],
                                 start=True, stop=True)
                nc.tensor.matmul(gps[:, 3 * H : 4 * H],
                                 lhsT=uk[:, 2 * H : 3 * H], rhs=h_bf[:],
                                 start=True, stop=True)
                # yz = tanh((mz + bz)/2) = 2*sigmoid(mz+bz) - 1
                zT = wp.tile([BLK, BLK], F32, tag="zT")
                nc.scalar.activation(zT[:], gps[:, 0:H],
                                     mybir.ActivationFunctionType.Tanh,
                                     bias=bkc[:, 0:1], scale=0.5)
                rT = wp.tile([BLK, BLK], F32, tag="rT")
                nc.scalar.activation(rT[:], gps[:, H : 2 * H],
                                     mybir.ActivationFunctionType.Tanh,
                                     bias=bkc[:, 1:2], scale=0.5)
                # hh = tanh(mhx + bhx + r*(mhh + bhh)),  r = (yr+1)/2
                # r*(mhh+bhh) = 0.5*(mhh+bhh)*yr + 0.5*(mhh+bhh)
                v_t = wp.tile([BLK, BLK], F32, tag="v_t")
                nc.vector.tensor_scalar(
                    out=v_t[:], in0=gps[:, 3 * H : 4 * H],
                    scalar1=bkc[:, 3:4], scalar2=0.5,
                    op0=mybir.AluOpType.add, op1=mybir.AluOpType.mult)
                t2 = wp.tile([BLK, BLK], F32, tag="t2")
                nc.vector.tensor_tensor(out=t2[:], in0=v_t[:], in1=rT[:],
                                        op=mybir.AluOpType.mult)
                t3 = wp.tile([BLK, BLK], F32, tag="t3")
                nc.vector.tensor_tensor(out=t3[:], in0=t2[:], in1=v_t[:],
                                        op=mybir.AluOpType.add)
                t4 = wp.tile([BLK, BLK], F32, tag="v_t")
                nc.vector.tensor_tensor(out=t4[:], in0=t3[:],
                                        in1=gps[:, 2 * H : 3 * H],
                                        op=mybir.AluOpType.add)
                hhT = wp.tile([BLK, BLK], F32, tag="hhT")
                nc.scalar.activation(hhT[:], t4[:],
                                     mybir.ActivationFunctionType.Tanh,
                                     bias=bkc[:, 2:3])
                # hn = hh + z*(h - hh),  z = (yz+1)/2
                #    = hh + 0.5*d + 0.5*yz*d,  d = h - hh
                d_t = wp.tile([BLK, BLK], F32, tag="d_t")
                nc.vector.tensor_tensor(out=d_t[:],
                                        in0=hT[:, bass.ts(b, BLK)], in1=hhT[:],
                                        op=mybir.AluOpType.subtract)
                e_t = wp.tile([BLK, BLK], F32, tag="zT")
                nc.vector.tensor_tensor(out=e_t[:], in0=zT[:], in1=d_t[:],
                                        op=mybir.AluOpType.mult)
                s_t = wp.tile([BLK, BLK], F32, tag="t2")
                nc.vector.tensor_tensor(out=s_t[:], in0=e_t[:], in1=d_t[:],
                                        op=mybir.AluOpType.add)
                nc.vector.scalar_tensor_tensor(
                    out=hT[:, bass.ts(b, BLK)], in0=s_t[:], scalar=0.5,
                    in1=hhT[:], op0=mybir.AluOpType.mult,
                    op1=mybir.AluOpType.add)
                if t < t_iters - 1:
                    ac_block(b)

            # --- prologue: A/C from h0, first AllGather ---
            for b in range(NBLK):
                ac_block(b)
            nc.gpsimd.collective_compute(
                "AllGather", mybir.AluOpType.bypass,
                replica_groups=AG_GROUPS,
                ins=[shardA.opt()], outs=[repA.opt()])

            def iteration(t):
                rep = repA if t % 2 == 0 else repB
                rep_next = repB if t % 2 == 0 else repA
                tabs = 0
                for gi, blocks in enumerate(groups):
                    nt = grp_ntiles[gi]
                    gs0 = grp_slot0[gi]
                    gm = gmp.tile([BLK, nt * BLK], BF16, tag="gm")
                    off = 0
                    for q in range(NQ):
                        csz = call_sizes[gi * NQ + q]
                        while csz > 0:
                            csub = min(csz, 1024)
                            out_ap = gm[:, off : off + csub].rearrange(
                                "p (k e) -> p k e", e=H)
                            nc.gpsimd.dma_gather(
                                out_ap,
                                rep[:, q * H : (q + 1) * H],
                                idx_sb[:, (gs0 + off) // 16 :
                                       (gs0 + off + csub) // 16],
                                csub, csub, H, elem_step=NQ * H)
                            off += csub
                            csz -= csub
                    indT_sb = indp.tile([BLK, nt * BLK], F8, tag="indT")
                    nc.sync.dma_start(indT_sb[:],
                                      indT_in[:, gs0 : gs0 + nt * BLK])
                    indm_sb = indp.tile([BLK, nt * BLK], F8, tag="indm")
                    nc.sync.dma_start(indm_sb[:],
                                      indm_in[:, gs0 : gs0 + nt * BLK])

                    # block-major compute order: each block's tiles are
                    # contiguous so its PSUM accumulation group is one
                    # uninterrupted start..stop span.
                    order = []
                    blk_first = {}
                    blk_last = {}
                    for b in blocks:
                        for q in range(NQ):
                            t0b = int(tile_base[b, q])
                            for k in range(int(P[b, q])):
                                ti = t0b + k
                                if b not in blk_first:
                                    blk_first[b] = ti
                                blk_last[b] = ti
                                order.append(ti)
                    aggof = {}
                    u = 0
                    while u < nt:
                        w = min(4, nt - u)
                        xps = psX.tile([BLK, 4 * BLK], F32, space="PSUM",
                                       tag="x")
                        for k in range(w):
                            ti = order[u + k]
                            tb = int(tile_block[ti])
                            sl = slice(ti * BLK - gs0, (ti + 1) * BLK - gs0)
                            nc.tensor.matmul(
                                xps[:, bass.ts(k, BLK)],
                                lhsT=indT_sb[:, sl],
                                rhs=c_all[:, bass.ts(tb, H)],
                                start=True, stop=False)
                            nc.tensor.matmul(
                                xps[:, bass.ts(k, BLK)],
                                lhsT=ident_b[:], rhs=gm[:, sl],
                                start=False, stop=True)
                        r_sb = spool.tile([BLK, 4 * BLK], BF16, tag="r")
                        nc.scalar.activation(
                            r_sb[:, : w * BLK], xps[:, : w * BLK],
                            mybir.ActivationFunctionType.Relu, scale=LAM)
                        e_sb = spool.tile([BLK, 4 * BLK], BF16, tag="e")
                        nc.scalar.activation(
                            e_sb[:, : w * BLK], xps[:, : w * BLK],
                            mybir.ActivationFunctionType.Exp,
                            bias=lnla_c[:, :1], scale=1.0)
                        b_sb = spool.tile([BLK, 4 * BLK], BF16, tag="b")
                        nc.vector.tensor_scalar(
                            out=b_sb[:, : w * BLK], in0=e_sb[:, : w * BLK],
                            scalar1=LA, scalar2=0.0,
                            op0=mybir.AluOpType.subtract,
                            op1=mybir.AluOpType.min)
                        for k in range(w):
                            ti = order[u + k]
                            tb = int(tile_block[ti])
                            sl = slice(ti * BLK - gs0, (ti + 1) * BLK - gs0)
                            if blk_first[tb] == ti:
                                agg_t = psA.tile(
                                    [BLK, BLK], F32, space="PSUM", tag="agg")
                                aggof[tb] = agg_t
                            nc.tensor.matmul(
                                aggof[tb][:],
                                lhsT=r_sb[:, bass.ts(k, BLK)],
                                rhs=indm_sb[:, sl],
                                start=bool(blk_first[tb] == ti), stop=False)
                            nc.tensor.matmul(
                                aggof[tb][:],
                                lhsT=b_sb[:, bass.ts(k, BLK)],
                                rhs=indm_sb[:, sl],
                                start=False, stop=bool(blk_last[tb] == ti))
                            if blk_last[tb] == ti:
                                gru_block(tb, aggof[tb][:], t)
                        u += w
                    tabs += nt
                if t < t_iters - 1:
                    nc.gpsimd.collective_compute(
                        "AllGather", mybir.AluOpType.bypass,
                        replica_groups=AG_GROUPS,
                        ins=[shardA.opt()], outs=[rep_next.opt()])

            for t in range(t_iters):
                iteration(t)

            # --- pooling ---
            pool_ps = psX.tile([BLK, G], F32, space="PSUM", tag="x")
            for b in range(NBLK):
                gps = psG.tile([BLK, 4 * H], F32, space="PSUM", tag="gru")
                nc.tensor.transpose(gps[:, 0:H], hT[:, bass.ts(b, BLK)],
                                    ident_f[:])
                hb_sb = fp.tile([BLK, BLK], BF16, tag="hb_sb")
                nc.vector.tensor_copy(hb_sb[:], gps[:, 0:H])
                indg = fp.tile([BLK, G], BF16, tag="indg")
                nc.vector.tensor_scalar(
                    out=indg[:], in0=iota_g[:], scalar1=gid_sb[:, b : b + 1],
                    scalar2=None, op0=mybir.AluOpType.is_equal)
                nc.tensor.matmul(pool_ps[:], lhsT=hb_sb[:], rhs=indg[:],
                                 start=(b == 0), stop=(b == NBLK - 1))
            pooledT = fp.tile([BLK, G], F32, tag="pooledT")
            nc.vector.tensor_copy(pooledT[:], pool_ps[:])
            nc.sync.dma_start(pool_in[:], pooledT[:])
            nc.gpsimd.collective_compute(
                "AllReduce", mybir.AluOpType.add,
                replica_groups=AG_GROUPS,
                ins=[pool_in.opt()], outs=[pool_out.opt()])
            pld_f = fp.tile([BLK, G], F32, tag="pooledT")
            nc.sync.dma_start(pld_f[:], pool_out[:])
            pld = fp.tile([BLK, G], BF16, tag="pld")
            nc.vector.tensor_copy(pld[:], pld_f[:])

            def selu_block(x_ps, brel_col, bexp_col, tagp):
                rr = fp.tile([BLK, G], BF16, tag="f_r")
                nc.scalar.activation(rr[:], x_ps[:],
                                     mybir.ActivationFunctionType.Relu,
                                     bias=brel_col, scale=LAM)
                ee = fp.tile([BLK, G], BF16, tag="f_e")
                nc.scalar.activation(ee[:], x_ps[:],
                                     mybir.ActivationFunctionType.Exp,
                                     bias=bexp_col, scale=1.0)
                bb = fp.tile([BLK, G], BF16, tag="f_b")
                nc.vector.tensor_scalar(
                    out=bb[:], in0=ee[:], scalar1=LA, scalar2=0.0,
                    op0=mybir.AluOpType.subtract, op1=mybir.AluOpType.min)
                oo = fp.tile([BLK, G], BF16, tag=f"{tagp}_o")
                nc.vector.tensor_tensor(out=oo[:], in0=rr[:], in1=bb[:],
                                        op=mybir.AluOpType.add)
                return oo

            x1 = []
            for half in range(2):
                x_ps = psX.tile([BLK, G], F32, space="PSUM", tag="x")
                nc.tensor.matmul(x_ps[:], lhsT=w1[:, bass.ts(half, BLK)],
                                 rhs=pld[:], start=True, stop=True)
                x1.append(selu_block(x_ps, b1r[:, half : half + 1],
                                     b1e[:, half : half + 1], f"x1{half}"))
            x2 = []
            for half in range(2):
                x_ps = psX.tile([BLK, G], F32, space="PSUM", tag="x")
                nc.tensor.matmul(x_ps[:], lhsT=w2q[0][half][:], rhs=x1[0][:],
                                 start=True, stop=False)
                nc.tensor.matmul(x_ps[:], lhsT=w2q[1][half][:], rhs=x1[1][:],
                                 start=False, stop=True)
                x2.append(selu_block(x_ps, b2r[:, half : half + 1],
                                     b2e[:, half : half + 1], f"x2{half}"))
            x3_ps = psG.tile([1, G], F32, space="PSUM", tag="gru")
            nc.tensor.matmul(x3_ps[:], lhsT=w3a[:], rhs=x2[0][:],
                             start=True, stop=False)
            nc.tensor.matmul(x3_ps[:], lhsT=w3b[:], rhs=x2[1][:],
                             start=False, stop=True)
            out_sb = fp.tile([1, G], F32, tag="out_sb")
            nc.scalar.activation(out_sb[:], x3_ps[:],
                                 mybir.ActivationFunctionType.Identity,
                                 bias=b3c[:1, :1])
            nc.sync.dma_start(out_dram[:], out_sb[:])

    nc.compile()
    return nc


def kernel(features, edges_topology, graph_ids, Wm, bm, Wk, Uk, bk,
           W1, b1, W2, b2, W3, b3, _trace=False, _t_iters=T_ITERS):
    features = np.asarray(features, np.float32)
    Wm = np.asarray(Wm, np.float32)
    bm = np.asarray(bm, np.float32)
    Wk = np.asarray(Wk, np.float32)
    Uk = np.asarray(Uk, np.float32)
    bk = np.asarray(bk, np.float32)
    W1 = np.asarray(W1, np.float32)
    b1 = np.asarray(b1, np.float32)
    W2 = np.asarray(W2, np.float32)
    b2 = np.asarray(b2, np.float32)
    W3 = np.asarray(W3, np.float32)
    b3 = np.asarray(b3, np.float32)
    et = np.asarray(edges_topology)

    meta = _preprocess(features, et[0], et[1], graph_ids)
    nc = _build_program(meta, float(b3[0]), t_iters=_t_iters)

    bkc = np.stack(
        [
            0.5 * (bk[0, 0:H] + bk[1, 0:H]),      # z gate: tanh(x/2 + b/2)
            0.5 * (bk[0, H : 2 * H] + bk[1, H : 2 * H]),  # r gate likewise
            bk[0, 2 * H : 3 * H],
            bk[1, 2 * H : 3 * H],
        ],
        axis=1,
    ).astype(np.float32)

    b1r = np.stack([LAM * b1[0:BLK], LAM * b1[BLK:RU]], axis=1).astype(np.float32)
    b1e = np.stack([b1[0:BLK] + LNLA, b1[BLK:RU] + LNLA], axis=1).astype(np.float32)
    b2r = np.stack([LAM * b2[0:BLK], LAM * b2[BLK:RU]], axis=1).astype(np.float32)
    b2e = np.stack([b2[0:BLK] + LNLA, b2[BLK:RU] + LNLA], axis=1).astype(np.float32)

    in_maps = []
    for c in range(N_CORES):
        in_maps.append(
            {
                "h0T": meta["h0T"][c],
                "idx16": meta["idx16"][c],
                "indT": meta["indT"][c],
                "indm": meta["indm"][c],
                "gid": meta["gid"][c],
                "wm1": np.ascontiguousarray(Wm[0:H]),
                "wm2": np.ascontiguousarray(Wm[H : 2 * H]),
                "bmr": bm.reshape(1, H),
                "wk": Wk,
                "uk": Uk,
                "bkc": bkc,
                "w1": W1,
                "w2": W2,
                "w3": W3,
                "b1r": b1r,
                "b1e": b1e,
                "b2r": b2r,
                "b2e": b2e,
            }
        )

    res = bass_utils.run_bass_kernel_spmd(
        nc, in_maps, core_ids=list(range(N_CORES)), trace=_trace
    )
    out = res.results[0]["out"].reshape(G, 1).astype(np.float32)
    kernel.last_results = res
    return out


# revision 33
# speedup vs baseline: 4.6898x; 1.0085x over previous
"""Trainium2 Bass kernel for nn_MessagePassingNN (GNN message passing).

Strategy (8 NeuronCores, SPMD), v2 -- see problem notes:
  - Nodes sharded by index range: core c owns nodes [c*12500, (c+1)*12500).
  - Per iteration each core computes per-node terms A = h@Wm1, C = h@Wm2+bm
    (bf16) for its shard, AllGathers A into a bf16 replica viewed as
    [25088, 512] (4 node-rows per 1024B physical row).
  - Messages msg = selu(A[first] + C[second]): edges grouped by destination
    128-node block and source-row residue q = row%4 (int16 dma_gather limit);
    each (block, q) run padded to 128-message tiles with a cross-core-uniform
    tile pattern P[b, q].  A[first] via one dma_gather per (4-block group, q);
    C[second] expanded per tile by an indicator matmul (indT, host-streamed);
    A added by identity matmul into the same PSUM bank (4 tiles per bank).
    selu = relu part (scalar engine) + exp part (scalar + vector min); both
    segment-summed into the group's agg PSUM bank via indicator matmuls
    (indm, host-streamed).
  - GRU per block (feature-transposed, bf16 matmuls / fp32 state), then A/C
    for the next iteration immediately.
  - Final: indicator-matmul graph pooling, AllReduce, 3-layer MLP, out[512,1].
"""
import sys

sys.path.insert(0, "/opt/trn_rl_repo")

import numpy as np
import ml_dtypes

import concourse.bass as bass
import concourse.bacc as bacc
import concourse.mybir as mybir
import concourse.tile as tile
import concourse.bass_utils as bass_utils
from concourse.masks import make_identity

F32 = mybir.dt.float32
BF16 = mybir.dt.bfloat16
F8 = mybir.dt.float8e4
I32 = mybir.dt.int32
I16 = mybir.dt.int16

N_NODES = 100000
N_CORES = 8
SH = N_NODES // N_CORES          # 12500
BLK = 128
NBLK = (SH + BLK - 1) // BLK     # 98
SHP = NBLK * BLK                 # 12544
H = 128
T_ITERS = 8
G = 512
RU = 256
NQ = 4
QROWS = (N_CORES * SHP) // NQ    # 25088
GB = 4                           # blocks per gather group
NGRP = (NBLK + GB - 1) // GB     # 25 groups (last has 2 blocks)

LAM = 1.0507009873554805
ALPHA = 1.6732632423543772
LA = LAM * ALPHA
LNLA = float(np.log(LA))

AG_GROUPS = [list(range(N_CORES))]


def _sm_row(node):
    node = np.asarray(node)
    return (node // SH) * SHP + (node % SH)


def _pack_core(deg4, rng):
    """Assign SH nodes (class = original position % NQ, fixed) to blocks so
    per-(block, src-residue) in-edge counts fit the alternating pattern
    P[b, q] = 2 + ((b + q) % 2) tiles.  Returns pos[n] = permuted position,
    preserving pos % NQ == n % NQ.  Greedy; degree caps may be exceeded (the
    caller recomputes the realized pattern)."""
    capP = np.array([[2 + ((b + q) % 2) for q in range(NQ)]
                     for b in range(NBLK)], np.int64) * BLK
    cls = np.arange(SH) % NQ
    cls_slots = np.full((NBLK, NQ), BLK // NQ, np.int64)
    cls_slots[-1] = (SH - BLK * (NBLK - 1)) // NQ        # 21 per class
    fill = np.zeros((NBLK, NQ), np.int64)
    order = np.argsort(-deg4.sum(1), kind="stable")
    assign = np.full(SH, -1, np.int64)
    BIG = 1 << 40
    for n in order:
        c = cls[n]
        d = deg4[n]
        rem = capP - fill - d                      # [NBLK, NQ]
        over = np.where(rem < 0, -rem, 0).sum(1)   # overflow if placed here
        room = rem.min(1)
        score = over * 4096 - room
        score[cls_slots[:, c] <= 0] = BIG
        b = int(np.argmin(score))
        fill[b] += d
        cls_slots[b, c] -= 1
        assign[n] = b
    # positions: class-c nodes of block b -> slots b*BLK + c, c+4, ...
    pos = np.full(SH, -1, np.int64)
    nxt = np.zeros((NBLK, NQ), np.int64)
    for n in range(SH):
        b, c = assign[n], cls[n]
        pos[n] = b * BLK + c + NQ * nxt[b, c]
        nxt[b, c] += 1
    return pos


def _preprocess(features, first, second, graph_ids):
    first = np.asarray(first, dtype=np.int64)
    second = np.asarray(second, dtype=np.int64)
    graph_ids = np.asarray(graph_ids, dtype=np.int64)
    features = np.asarray(features, dtype=np.float32)
    rng = np.random.default_rng(0)

    owner_src = first // SH
    owner = second // SH
    src_q0 = (first % SH) % NQ          # class: preserved by the permutation

    # per-core degree-aware packing (class-preserving permutation)
    pos_list = []
    for c in range(N_CORES):
        m = owner == c
        loc = second[m] - c * SH
        d4 = np.zeros((SH, NQ), np.int64)
        np.add.at(d4, (loc, src_q0[m]), 1)
        pos_list.append(_pack_core(d4, rng))

    # permuted source rows (per owner) -> gather idx and residue
    pos_all = np.concatenate(
        [o * SHP + pos_list[o] for o in range(N_CORES)])   # [N_NODES]
    src_row = pos_all[first]            # permuted sm_row of source
    src_q = (src_row % NQ).astype(np.int64)
    src_idx = (src_row // NQ).astype(np.int64)     # < 25088, int16-safe
    assert (src_q == src_q0).all()

    per_core = []
    counts = np.zeros((N_CORES, NBLK, NQ), np.int64)
    for c in range(N_CORES):
        m = owner == c
        loc = pos_list[c][second[m] - c * SH]
        blk = loc // BLK
        q = src_q[m]
        si = src_idx[m]
        order = np.lexsort((si, q, blk))
        per_core.append((loc[order], blk[order], q[order], si[order]))
        np.add.at(counts[c], (blk, q), 1)

    P = np.maximum(1, -(-counts.max(axis=0) // BLK))      # [NBLK, NQ] tiles

    groups = [list(range(g * GB, min((g + 1) * GB, NBLK))) for g in range(NGRP)]
    tile_block = []
    call_sizes = []
    for blocks in groups:
        for q in range(NQ):
            csz = 0
            for b in blocks:
                for _ in range(int(P[b, q])):
                    tile_block.append(b)
                    csz += BLK
            call_sizes.append(csz)
    TT = len(tile_block)
    LC = TT * BLK
    tile_block = np.array(tile_block, np.int64)
    first_tile = np.full(NBLK, -1, np.int64)
    last_tile = np.zeros(NBLK, np.int64)
    for t, b in enumerate(tile_block):
        if first_tile[b] < 0:
            first_tile[b] = t
        last_tile[b] = t

    # slot base per (b, q)
    slot_base = np.zeros((NBLK, NQ), np.int64)
    s = 0
    for blocks in groups:
        for q in range(NQ):
            for b in blocks:
                slot_base[b, q] = s
                s += int(P[b, q]) * BLK
    assert s == LC

    idx16_list, indT_list, indm_list = [], [], []
    h0T_list, gid_list = [], []
    for c in range(N_CORES):
        loc, blk, q, sidx = per_core[c]
        seg = np.full(LC, -1.0, np.float32)
        gidx = np.zeros(LC, np.int64)
        if len(blk):
            run_change = np.ones(len(blk), bool)
            run_change[1:] = (blk[1:] != blk[:-1]) | (q[1:] != q[:-1])
            run_start = np.maximum.accumulate(
                np.where(run_change, np.arange(len(blk)), 0))
            within = np.arange(len(blk)) - run_start
            slot = slot_base[blk, q] + within
            seg[slot] = (loc - blk * BLK).astype(np.float32)
            gidx[slot] = sidx
        idx16 = gidx.astype(np.int16)
        idxw = np.zeros((16, LC // 16), np.int16)
        s0 = 0
        for csz in call_sizes:
            if csz:
                idxw[:, s0 // 16 : (s0 + csz) // 16] = (
                    idx16[s0 : s0 + csz].reshape(csz // 16, 16).T)
                s0 += csz
        idx_full = np.ascontiguousarray(np.tile(idxw, (8, 1)))
        segs = seg.reshape(TT, 1, BLK)
        dd = np.arange(BLK, dtype=np.float32).reshape(1, BLK, 1)
        indT = (segs == dd).astype(ml_dtypes.float8_e4m3)   # [TT, dest, msg]
        indT_h = np.ascontiguousarray(
            indT.transpose(1, 0, 2).reshape(BLK, LC))
        indm = indT.transpose(0, 2, 1)                   # [TT, msg, dest]
        indm_h = np.ascontiguousarray(
            indm.transpose(1, 0, 2).reshape(BLK, LC))
        idx16_list.append(idx_full)
        indT_list.append(indT_h)
        indm_list.append(indm_h)
        h0 = np.zeros((SHP, H), np.float32)
        h0[pos_list[c]] = features[c * SH : (c + 1) * SH]
        h0T_list.append(np.ascontiguousarray(h0.T))
        gid = np.full(SHP, -1.0, np.float32)
        gid[pos_list[c]] = graph_ids[c * SH : (c + 1) * SH].astype(np.float32)
        gid_list.append(np.ascontiguousarray(gid.reshape(NBLK, BLK).T))

    return dict(
        P=P, groups=groups, TT=TT, LC=LC, call_sizes=call_sizes,
        tile_block=tile_block, first_tile=first_tile, last_tile=last_tile,
        tile_base=slot_base // BLK,
        idx16=idx16_list, indT=indT_list, indm=indm_list,
        h0T=h0T_list, gid=gid_list,
    )


def _build_program(meta, b3_val, t_iters=T_ITERS):
    groups = meta["groups"]
    TT = meta["TT"]
    LC = meta["LC"]
    call_sizes = meta["call_sizes"]
    tile_block = meta["tile_block"]
    tile_base = meta["tile_base"]
    P = meta["P"]

    nc = bacc.Bacc(
        "TRN2",
        target_bir_lowering=False,
        debug=False,
        enable_asserts=False,
        num_devices=N_CORES,
    )

    h0T_in = nc.dram_tensor("h0T", [BLK, SHP], F32, kind="ExternalInput")
    idx_in = nc.dram_tensor("idx16", [BLK, LC // 16], I16, kind="ExternalInput")
    indT_in = nc.dram_tensor("indT", [BLK, LC], F8, kind="ExternalInput")
    indm_in = nc.dram_tensor("indm", [BLK, LC], F8, kind="ExternalInput")
    gid_in = nc.dram_tensor("gid", [BLK, NBLK], F32, kind="ExternalInput")
    wm1_in = nc.dram_tensor("wm1", [H, H], F32, kind="ExternalInput")
    wm2_in = nc.dram_tensor("wm2", [H, H], F32, kind="ExternalInput")
    bmr_in = nc.dram_tensor("bmr", [1, H], F32, kind="ExternalInput")
    wk_in = nc.dram_tensor("wk", [H, 3 * H], F32, kind="ExternalInput")
    uk_in = nc.dram_tensor("uk", [H, 3 * H], F32, kind="ExternalInput")
    bkc_in = nc.dram_tensor("bkc", [BLK, 4], F32, kind="ExternalInput")
    w1_in = nc.dram_tensor("w1", [H, RU], F32, kind="ExternalInput")
    w2_in = nc.dram_tensor("w2", [RU, RU], F32, kind="ExternalInput")
    w3_in = nc.dram_tensor("w3", [RU, 1], F32, kind="ExternalInput")
    b1r_in = nc.dram_tensor("b1r", [BLK, 2], F32, kind="ExternalInput")
    b1e_in = nc.dram_tensor("b1e", [BLK, 2], F32, kind="ExternalInput")
    b2r_in = nc.dram_tensor("b2r", [BLK, 2], F32, kind="ExternalInput")
    b2e_in = nc.dram_tensor("b2e", [BLK, 2], F32, kind="ExternalInput")
    out_dram = nc.dram_tensor("out", [1, G], F32, kind="ExternalOutput")

    # per-group slot ranges
    grp_slot0, grp_ntiles = [], []
    s0 = 0
    ci = 0
    for blocks in groups:
        n = 0
        for q in range(NQ):
            n += call_sizes[ci]
            ci += 1
        grp_slot0.append(s0)
        grp_ntiles.append(n // BLK)
        s0 += n

    with tile.TileContext(nc) as tc:
        with (
            tc.tile_pool(name="const", bufs=1) as cp,
            tc.tile_pool(name="gmp", bufs=2) as gmp,
            tc.tile_pool(name="indp", bufs=2) as indp,
            tc.tile_pool(name="sp", bufs=4) as spool,
            tc.tile_pool(name="wp", bufs=2) as wp,
            tc.tile_pool(name="final", bufs=1) as fp,
            tc.tile_pool(name="ps_x", bufs=2, space="PSUM") as psX,
            tc.tile_pool(name="ps_agg", bufs=2, space="PSUM") as psA,
            tc.tile_pool(name="ps_gru", bufs=2, space="PSUM") as psG,
            tc.tile_pool(name="ps_ac", bufs=2, space="PSUM") as psC,
            tc.tile_pool(name="dram", bufs=1, space="DRAM") as dp,
        ):
            ident_b = cp.tile([BLK, BLK], BF16)
            make_identity(nc, ident_b[:])
            ident_f = cp.tile([BLK, BLK], F32)
            make_identity(nc, ident_f[:])
            iota_g = cp.tile([BLK, G], F32)
            lnla_c = cp.tile([BLK, 1], F32)
            nc.gpsimd.memset(lnla_c[:], LNLA)
            ones1_b = cp.tile([1, BLK], BF16)
            nc.gpsimd.memset(ones1_b[:], 1.0)

            iog_i = wp.tile([BLK, G], I32, tag="iogi")
            nc.gpsimd.iota(iog_i[:], pattern=[[1, G]], base=0,
                           channel_multiplier=0)
            nc.vector.tensor_copy(iota_g[:], iog_i[:])
            hT = cp.tile([BLK, SHP], F32)
            nc.sync.dma_start(hT[:], h0T_in[:])
            idx_sb = cp.tile([BLK, LC // 16], I16)
            nc.sync.dma_start(idx_sb[:], idx_in[:])
            gid_sb = cp.tile([BLK, NBLK], F32)
            nc.sync.dma_start(gid_sb[:], gid_in[:])

            def load_bf(t_in, shape, tag):
                t32 = wp.tile(shape, F32, tag="ldf32")
                nc.sync.dma_start(t32[:], t_in[:])
                tb = cp.tile(shape, BF16, tag=tag)
                nc.vector.tensor_copy(tb[:], t32[:])
                return tb

            wm1 = load_bf(wm1_in, [H, H], "wm1")
            wm2 = load_bf(wm2_in, [H, H], "wm2")
            wk = load_bf(wk_in, [H, 3 * H], "wk")
            uk = load_bf(uk_in, [H, 3 * H], "uk")
            w1 = load_bf(w1_in, [H, RU], "w1")
            bmr_b = load_bf(bmr_in, [1, H], "bmr")
            bkc = cp.tile([BLK, 4], F32)
            nc.sync.dma_start(bkc[:], bkc_in[:])
            w2q = []
            for i in range(2):
                row = []
                for j in range(2):
                    t32 = wp.tile([BLK, BLK], F32, tag="ldw2")
                    nc.sync.dma_start(
                        t32[:], w2_in[bass.ts(i, BLK), bass.ts(j, BLK)])
                    tb = cp.tile([BLK, BLK], BF16, tag=f"w2q{i}{j}")
                    nc.vector.tensor_copy(tb[:], t32[:])
                    row.append(tb)
                w2q.append(row)
            w3ab = []
            for i in range(2):
                t32 = wp.tile([BLK, 1], F32, tag="ldw3")
                nc.sync.dma_start(t32[:], w3_in[bass.ts(i, BLK), :])
                tb = cp.tile([BLK, 1], BF16, tag=f"w3{i}")
                nc.vector.tensor_copy(tb[:], t32[:])
                w3ab.append(tb)
            w3a, w3b = w3ab
            b1r = cp.tile([BLK, 2], F32)
            nc.sync.dma_start(b1r[:], b1r_in[:])
            b1e = cp.tile([BLK, 2], F32)
            nc.sync.dma_start(b1e[:], b1e_in[:])
            b2r = cp.tile([BLK, 2], F32)
            nc.sync.dma_start(b2r[:], b2r_in[:])
            b2e = cp.tile([BLK, 2], F32)
            nc.sync.dma_start(b2e[:], b2e_in[:])
            b3c = cp.tile([1, 1], F32)
            nc.gpsimd.memset(b3c[:], float(b3_val))

            c_all = cp.tile([BLK, NBLK * H], BF16)

            repA = dp.tile([QROWS, NQ * H], BF16)
            repB = dp.tile([QROWS, NQ * H], BF16)
            shardA = dp.tile([SHP, H], BF16)
            pool_in = dp.tile([BLK, G], F32)
            pool_out = dp.tile([BLK, G], F32)

            def ac_block(b):
                h_bf = wp.tile([BLK, BLK], BF16, tag="h_bf")
                nc.vector.tensor_copy(h_bf[:], hT[:, bass.ts(b, BLK)])
                ac_ps = psC.tile([BLK, 2 * H], F32, space="PSUM", tag="ac")
                nc.tensor.matmul(ac_ps[:, 0:H], lhsT=h_bf[:], rhs=wm1[:],
                                 start=True, stop=True)
                nc.tensor.matmul(ac_ps[:, H : 2 * H], lhsT=h_bf[:], rhs=wm2[:],
                                 start=True, stop=False)
                nc.tensor.matmul(ac_ps[:, H : 2 * H], lhsT=ones1_b[:],
                                 rhs=bmr_b[:], start=False, stop=True)
                a_sb = wp.tile([BLK, H], BF16, tag="a_sb")
                nc.vector.tensor_copy(a_sb[:], ac_ps[:, 0:H])
                nc.sync.dma_start(shardA[bass.ts(b, BLK), :], a_sb[:])
                nc.vector.tensor_copy(c_all[:, bass.ts(b, H)],
                                      ac_ps[:, H : 2 * H])

            def gru_block(b, agg_ap, t):
                # sigmoid(x) = (tanh(x/2) + 1)/2 so every activation comes
                # from the exp_and_others table set (no table reloads).
                aggb = wp.tile([BLK, BLK], BF16, tag="aggb")
                nc.vector.tensor_copy(aggb[:], agg_ap)
                h_bf = wp.tile([BLK, BLK], BF16, tag="hgru")
                nc.vector.tensor_copy(h_bf[:], hT[:, bass.ts(b, BLK)])
                gps = psG.tile([BLK, 4 * H], F32, space="PSUM", tag="gru")
                nc.tensor.matmul(gps[:, 0:H], lhsT=wk[:, 0:H], rhs=aggb[:],
                                 start=True, stop=False)
                nc.tensor.matmul(gps[:, 0:H], lhsT=uk[:, 0:H], rhs=h_bf[:],
                                 start=False, stop=True)
                nc.tensor.matmul(gps[:, H : 2 * H], lhsT=wk[:, H : 2 * H],
                                 rhs=aggb[:], start=True, stop=False)
                nc.tensor.matmul(gps[:, H : 2 * H], lhsT=uk[:, H : 2 * H],
                                 rhs=h_bf[:], start=False, stop=True)
                nc.tensor.matmul(gps[:, 2 * H : 3 * H],
                                 lhsT=wk[:, 2 * H : 3 * H], rhs=aggb[:],
                                 start=True, stop=True)
                nc.tensor.matmul(gps[:, 3 * H : 4 * H],
                                 lhsT=uk[:, 2 * H : 3 * H], rhs=h_bf[:],
                                 start=True, stop=True)
                # yz = tanh((mz + bz)/2) = 2*sigmoid(mz+bz) - 1
                zT = wp.tile([BLK, BLK], F32, tag="zT")
                nc.scalar.activation(zT[:], gps[:, 0:H],
                                     mybir.ActivationFunctionType.Tanh,
                                     bias=bkc[:, 0:1], scale=0.5)
                rT = wp.tile([BLK, BLK], F32, tag="rT")
                nc.scalar.activation(rT[:], gps[:, H : 2 * H],
                                     mybir.ActivationFunctionType.Tanh,
                                     bias=bkc[:, 1:2], scale=0.5)
                # hh = tanh(mhx + bhx + r*(mhh + bhh)),  r = (yr+1)/2
                #    = tanh(0.5*((mhh+bhh)*yr) + (0.5*mhh + mhx)
                #           + (bhx + 0.5*bhh))
                u1 = wp.tile([BLK, BLK], F32, tag="v_t")
                nc.vector.scalar_tensor_tensor(
                    out=u1[:], in0=gps[:, 3 * H : 4 * H],
                    scalar=bkc[:, 3:4], in1=rT[:],
                    op0=mybir.AluOpType.add, op1=mybir.AluOpType.mult)
                v2 = wp.tile([BLK, BLK], F32, tag="t2")
                nc.vector.scalar_tensor_tensor(
                    out=v2[:], in0=u1[:], scalar=0.5,
                    in1=gps[:, 2 * H : 3 * H],
                    op0=mybir.AluOpType.mult, op1=mybir.AluOpType.add)
                t4 = wp.tile([BLK, BLK], F32, tag="t3")
                nc.vector.scalar_tensor_tensor(
                    out=t4[:], in0=gps[:, 3 * H : 4 * H], scalar=0.5,
                    in1=v2[:],
                    op0=mybir.AluOpType.mult, op1=mybir.AluOpType.add)
                hhT = wp.tile([BLK, BLK], F32, tag="hhT")
                nc.scalar.activation(hhT[:], t4[:],
                                     mybir.ActivationFunctionType.Tanh,
                                     bias=bkc[:, 2:3])
                # hn = hh + z*(h - hh) = hh + 0.5*(1+yz)*d,  d = h - hh
                d_t = wp.tile([BLK, BLK], F32, tag="d_t")
                nc.vector.tensor_tensor(out=d_t[:],
                                        in0=hT[:, bass.ts(b, BLK)], in1=hhT[:],
                                        op=mybir.AluOpType.subtract)
                p2 = wp.tile([BLK, BLK], F32, tag="zT")
                nc.vector.scalar_tensor_tensor(
                    out=p2[:], in0=zT[:], scalar=1.0, in1=d_t[:],
                    op0=mybir.AluOpType.add, op1=mybir.AluOpType.mult)
                nc.vector.scalar_tensor_tensor(
                    out=hT[:, bass.ts(b, BLK)], in0=p2[:], scalar=0.5,
                    in1=hhT[:], op0=mybir.AluOpType.mult,
                    op1=mybir.AluOpType.add)
                if t < t_iters - 1:
                    ac_block(b)

            # --- prologue: A/C from h0, first AllGather ---
            for b in range(NBLK):
                ac_block(b)
            nc.gpsimd.collective_compute(
                "AllGather", mybir.AluOpType.bypass,
                replica_groups=AG_GROUPS,
                ins=[shardA.opt()], outs=[repA.opt()])

            def iteration(t):
                rep = repA if t % 2 == 0 else repB
                rep_next = repB if t % 2 == 0 else repA
                tabs = 0
                for gi, blocks in enumerate(groups):
                    nt = grp_ntiles[gi]
                    gs0 = grp_slot0[gi]
                    gm = gmp.tile([BLK, nt * BLK], BF16, tag="gm")
                    off = 0
                    for q in range(NQ):
                        csz = call_sizes[gi * NQ + q]
                        while csz > 0:
                            csub = min(csz, 1024)
                            out_ap = gm[:, off : off + csub].rearrange(
                                "p (k e) -> p k e", e=H)
                            nc.gpsimd.dma_gather(
                                out_ap,
                                rep[:, q * H : (q + 1) * H],
                                idx_sb[:, (gs0 + off) // 16 :
                                       (gs0 + off + csub) // 16],
                                csub, csub, H, elem_step=NQ * H)
                            off += csub
                            csz -= csub
                    indT_sb = indp.tile([BLK, nt * BLK], F8, tag="indT")
                    nc.sync.dma_start(indT_sb[:],
                                      indT_in[:, gs0 : gs0 + nt * BLK])
                    indm_sb = indp.tile([BLK, nt * BLK], F8, tag="indm")
                    nc.sync.dma_start(indm_sb[:],
                                      indm_in[:, gs0 : gs0 + nt * BLK])

                    # block-major compute order: each block's tiles are
                    # contiguous so its PSUM accumulation group is one
                    # uninterrupted start..stop span.
                    order = []
                    blk_first = {}
                    blk_last = {}
                    for b in blocks:
                        for q in range(NQ):
                            t0b = int(tile_base[b, q])
                            for k in range(int(P[b, q])):
                                ti = t0b + k
                                if b not in blk_first:
                                    blk_first[b] = ti
                                blk_last[b] = ti
                                order.append(ti)
                    aggof = {}
                    u = 0
                    while u < nt:
                        w = min(4, nt - u)
                        xps = psX.tile([BLK, 4 * BLK], F32, space="PSUM",
                                       tag="x")
                        for k in range(w):
                            ti = order[u + k]
                            tb = int(tile_block[ti])
                            sl = slice(ti * BLK - gs0, (ti + 1) * BLK - gs0)
                            nc.tensor.matmul(
                                xps[:, bass.ts(k, BLK)],
                                lhsT=ident_b[:], rhs=gm[:, sl],
                                start=True, stop=False)
                            nc.tensor.matmul(
                                xps[:, bass.ts(k, BLK)],
                                lhsT=indT_sb[:, sl],
                                rhs=c_all[:, bass.ts(tb, H)],
                                start=False, stop=True)
                        e_sb = spool.tile([BLK, 4 * BLK], BF16, tag="e")
                        nc.scalar.activation(
                            e_sb[:, : w * BLK], xps[:, : w * BLK],
                            mybir.ActivationFunctionType.Exp,
                            bias=lnla_c[:, :1], scale=1.0)
                        r_sb = spool.tile([BLK, 4 * BLK], BF16, tag="r")
                        nc.scalar.activation(
                            r_sb[:, : w * BLK], xps[:, : w * BLK],
                            mybir.ActivationFunctionType.Relu, scale=LAM)
                        b_sb = spool.tile([BLK, 4 * BLK], BF16, tag="b")
                        nc.vector.tensor_scalar(
                            out=b_sb[:, : w * BLK], in0=e_sb[:, : w * BLK],
                            scalar1=LA, scalar2=0.0,
                            op0=mybir.AluOpType.subtract,
                            op1=mybir.AluOpType.min)
                        for k in range(w):
                            ti = order[u + k]
                            tb = int(tile_block[ti])
                            sl = slice(ti * BLK - gs0, (ti + 1) * BLK - gs0)
                            if blk_first[tb] == ti:
                                agg_t = psA.tile(
                                    [BLK, BLK], F32, space="PSUM", tag="agg")
                                aggof[tb] = agg_t
                            nc.tensor.matmul(
                                aggof[tb][:],
                                lhsT=r_sb[:, bass.ts(k, BLK)],
                                rhs=indm_sb[:, sl],
                                start=bool(blk_first[tb] == ti), stop=False)
                            nc.tensor.matmul(
                                aggof[tb][:],
                                lhsT=b_sb[:, bass.ts(k, BLK)],
                                rhs=indm_sb[:, sl],
                                start=False, stop=bool(blk_last[tb] == ti))
                            if blk_last[tb] == ti:
                                gru_block(tb, aggof[tb][:], t)
                        u += w
                    tabs += nt
                if t < t_iters - 1:
                    nc.gpsimd.collective_compute(
                        "AllGather", mybir.AluOpType.bypass,
                        replica_groups=AG_GROUPS,
                        ins=[shardA.opt()], outs=[rep_next.opt()])

            for t in range(t_iters):
                iteration(t)

            # --- pooling ---
            pool_ps = psX.tile([BLK, G], F32, space="PSUM", tag="x")
            for b in range(NBLK):
                gps = psG.tile([BLK, 4 * H], F32, space="PSUM", tag="gru")
                nc.tensor.transpose(gps[:, 0:H], hT[:, bass.ts(b, BLK)],
                                    ident_f[:])
                hb_sb = fp.tile([BLK, BLK], BF16, tag="hb_sb")
                nc.vector.tensor_copy(hb_sb[:], gps[:, 0:H])
                indg = fp.tile([BLK, G], BF16, tag="indg")
                nc.vector.tensor_scalar(
                    out=indg[:], in0=iota_g[:], scalar1=gid_sb[:, b : b + 1],
                    scalar2=None, op0=mybir.AluOpType.is_equal)
                nc.tensor.matmul(pool_ps[:], lhsT=hb_sb[:], rhs=indg[:],
                                 start=(b == 0), stop=(b == NBLK - 1))
            pooledT = fp.tile([BLK, G], F32, tag="pooledT")
            nc.vector.tensor_copy(pooledT[:], pool_ps[:])
            nc.sync.dma_start(pool_in[:], pooledT[:])
            nc.gpsimd.collective_compute(
                "AllReduce", mybir.AluOpType.add,
                replica_groups=AG_GROUPS,
                ins=[pool_in.opt()], outs=[pool_out.opt()])
            pld_f = fp.tile([BLK, G], F32, tag="pooledT")
            nc.sync.dma_start(pld_f[:], pool_out[:])
            pld = fp.tile([BLK, G], BF16, tag="pld")
            nc.vector.tensor_copy(pld[:], pld_f[:])

            def selu_block(x_ps, brel_col, bexp_col, tagp):
                rr = fp.tile([BLK, G], BF16, tag="f_r")
                nc.scalar.activation(rr[:], x_ps[:],
                                     mybir.ActivationFunctionType.Relu,
                                     bias=brel_col, scale=LAM)
                ee = fp.tile([BLK, G], BF16, tag="f_e")
                nc.scalar.activation(ee[:], x_ps[:],
                                     mybir.ActivationFunctionType.Exp,
                                     bias=bexp_col, scale=1.0)
                bb = fp.tile([BLK, G], BF16, tag="f_b")
                nc.vector.tensor_scalar(
                    out=bb[:], in0=ee[:], scalar1=LA, scalar2=0.0,
                    op0=mybir.AluOpType.subtract, op1=mybir.AluOpType.min)
                oo = fp.tile([BLK, G], BF16, tag=f"{tagp}_o")
                nc.vector.tensor_tensor(out=oo[:], in0=rr[:], in1=bb[:],
                                        op=mybir.AluOpType.add)
                return oo

            x1 = []
            for half in range(2):
                x_ps = psX.tile([BLK, G], F32, space="PSUM", tag="x")
                nc.tensor.matmul(x_ps[:], lhsT=w1[:, bass.ts(half, BLK)],
                                 rhs=pld[:], start=True, stop=True)
                x1.append(selu_block(x_ps, b1r[:, half : half + 1],
                                     b1e[:, half : half + 1], f"x1{half}"))
            x2 = []
            for half in range(2):
                x_ps = psX.tile([BLK, G], F32, space="PSUM", tag="x")
                nc.tensor.matmul(x_ps[:], lhsT=w2q[0][half][:], rhs=x1[0][:],
                                 start=True, stop=False)
                nc.tensor.matmul(x_ps[:], lhsT=w2q[1][half][:], rhs=x1[1][:],
                                 start=False, stop=True)
                x2.append(selu_block(x_ps, b2r[:, half : half + 1],
                                     b2e[:, half : half + 1], f"x2{half}"))
            x3_ps = psG.tile([1, G], F32, space="PSUM", tag="gru")
            nc.tensor.matmul(x3_ps[:], lhsT=w3a[:], rhs=x2[0][:],
                             start=True, stop=False)
            nc.tensor.matmul(x3_ps[:], lhsT=w3b[:], rhs=x2[1][:],
                             start=False, stop=True)
            out_sb = fp.tile([1, G], F32, tag="out_sb")
            nc.scalar.activation(out_sb[:], x3_ps[:],
                                 mybir.ActivationFunctionType.Identity,
                                 bias=b3c[:1, :1])
            nc.sync.dma_start(out_dram[:], out_sb[:])

    nc.compile()
    return nc


def kernel(features, edges_topology, graph_ids, Wm, bm, Wk, Uk, bk,
           W1, b1, W2, b2, W3, b3, _trace=False, _t_iters=T_ITERS):
    features = np.asarray(features, np.float32)
    Wm = np.asarray(Wm, np.float32)
    bm = np.asarray(bm, np.float32)
    Wk = np.asarray(Wk, np.float32)
    Uk = np.asarray(Uk, np.float32)
    bk = np.asarray(bk, np.float32)
    W1 = np.asarray(W1, np.float32)
    b1 = np.asarray(b1, np.float32)
    W2 = np.asarray(W2, np.float32)
    b2 = np.asarray(b2, np.float32)
    W3 = np.asarray(W3, np.float32)
    b3 = np.asarray(b3, np.float32)
    et = np.asarray(edges_topology)

    meta = _preprocess(features, et[0], et[1], graph_ids)
    nc = _build_program(meta, float(b3[0]), t_iters=_t_iters)

    bkc = np.stack(
        [
            0.5 * (bk[0, 0:H] + bk[1, 0:H]),      # z gate: tanh(x/2 + b/2)
            0.5 * (bk[0, H : 2 * H] + bk[1, H : 2 * H]),  # r gate likewise
            bk[0, 2 * H : 3 * H] + 0.5 * bk[1, 2 * H : 3 * H],  # tanh bias
            bk[1, 2 * H : 3 * H],
        ],
        axis=1,
    ).astype(np.float32)

    b1r = np.stack([LAM * b1[0:BLK], LAM * b1[BLK:RU]], axis=1).astype(np.float32)
    b1e = np.stack([b1[0:BLK] + LNLA, b1[BLK:RU] + LNLA], axis=1).astype(np.float32)
    b2r = np.stack([LAM * b2[0:BLK], LAM * b2[BLK:RU]], axis=1).astype(np.float32)
    b2e = np.stack([b2[0:BLK] + LNLA, b2[BLK:RU] + LNLA], axis=1).astype(np.float32)

    in_maps = []
    for c in range(N_CORES):
        in_maps.append(
            {
                "h0T": meta["h0T"][c],
                "idx16": meta["idx16"][c],
                "indT": meta["indT"][c],
                "indm": meta["indm"][c],
                "gid": meta["gid"][c],
                "wm1": np.ascontiguousarray(Wm[0:H]),
                "wm2": np.ascontiguousarray(Wm[H : 2 * H]),
                "bmr": bm.reshape(1, H),
                "wk": Wk,
                "uk": Uk,
                "bkc": bkc,
                "w1": W1,
                "w2": W2,
                "w3": W3,
                "b1r": b1r,
                "b1e": b1e,
                "b2r": b2r,
                "b2e": b2e,
            }
        )

    res = bass_utils.run_bass_kernel_spmd(
        nc, in_maps, core_ids=list(range(N_CORES)), trace=_trace
    )
    out = res.results[0]["out"].reshape(G, 1).astype(np.float32)
    kernel.last_results = res
    return out
